# revision 3
# baseline (speedup 1.0000x reference)
"""Trainium2 Bass kernel v3 for nn_MultiHeadAttention (B=2, S=4096, D=512, H=8).

Sharding: 8 cores = 2 batches x 4 query-slices of 1024 rows (no collectives).
Host prep compacts keys/values to the mask's nonzero slots (padded keys
contribute exp(bias)=const to the softmax denominator, subtracted via the
host-computed `npadn` scalar instead of a mask bias).

ACT-bound design (~134us exp floor on the scalar engine):
  - per head, kTr/qTr are rearranged by DMA so kt-even tiles live in
    partitions 0-63 and kt-odd in 64-127 (q replicated). Score matmuls for a
    kt-pair then occupy disjoint PE row-groups (tile_position (0,0)/(64,0))
    and execute concurrently (~2x effective PE rate at K=dh=64).
  - exp -> P2 [128,2,1024] fp8e4m3 per kt-pair, bias=-2 keeps P < e4m3 max.
  - pv: fp8 DoubleRow matmul over the kt-pair (contraction 256) into
    OT [65,1024] f32; the ones-column accumulates the softmax denominator.
  - emission is software-pipelined: each head's normalization is deferred
    into the next head's kt-loop, and block c+1's projections + rearranges
    are dribbled through block c's second-head kt-loop, so the scalar
    engine never waits at phase boundaries.
"""

import sys

if "/opt/trn_rl_repo" not in sys.path:
    sys.path.insert(0, "/opt/trn_rl_repo")

import numpy as np

import concourse.bass as bass  # noqa: F401
import concourse.mybir as mybir
import concourse.tile as tile
from concourse import bacc
from concourse.bass_utils import run_bass_kernel_spmd

F32 = mybir.dt.float32
BF16 = mybir.dt.bfloat16
FP8 = mybir.dt.float8e4  # only used when opts contains "dr"
EXP = mybir.ActivationFunctionType.Exp
DR = mybir.MatmulPerfMode.DoubleRow

B, S, D, H = 2, 4096, 512, 8
DH = D // H  # 64
NCORES = 8
QSPLIT = 4
QL = S // QSPLIT  # 1024
NJ = D // 128  # 4
EXP_SHIFT = -2.0  # keeps max P ~ exp(5.7-2) well under e4m3 max 240

_nc_cache: dict = {}


def _ceil_div(a, b):
    return (a + b - 1) // b


def build_program3(NKT, reps=1, opts=()):
    opts = set(opts)
    KP = NKT * 128
    NPAIR = NKT // 2
    TAIL = NKT % 2
    NE = NPAIR + TAIL  # kt-even tiles in kTr
    MDT = BF16
    USE_DR = "dr" in opts
    PV_DT = FP8 if USE_DR else BF16

    nc = bacc.Bacc(
        "TRN2", target_bir_lowering=False, debug=False, num_devices=NCORES
    )

    qT_d = nc.dram_tensor("qT", [D, QL], MDT, kind="ExternalInput").ap()
    kT_d = nc.dram_tensor("kT", [D, KP], MDT, kind="ExternalInput").ap()
    vT_d = nc.dram_tensor("vT", [D, KP], MDT, kind="ExternalInput").ap()
    Wq_d = nc.dram_tensor("Wqr", [D, H * 128], MDT, kind="ExternalInput").ap()
    Wk_d = nc.dram_tensor("Wkr", [D, H * 128], MDT, kind="ExternalInput").ap()
    Wv_d = nc.dram_tensor("Wv", [D, D], MDT, kind="ExternalInput").ap()
    Wo_d = nc.dram_tensor("Wo", [D, D], MDT, kind="ExternalInput").ap()
    bq_d = nc.dram_tensor("bq2r", [128, H], F32, kind="ExternalInput").ap()
    bk_d = nc.dram_tensor("bk2r", [128, H], F32, kind="ExternalInput").ap()
    bvb_d = nc.dram_tensor("bvb", [128, D], F32, kind="ExternalInput").ap()
    bob_d = nc.dram_tensor("bob", [128, D], F32, kind="ExternalInput").ap()
    npad_d = nc.dram_tensor("npadn", [1, 1], F32, kind="ExternalInput").ap()
    out_d = nc.dram_tensor("out", [QL, D], F32, kind="ExternalOutput").ap()

    with tile.TileContext(nc) as tc, \
         nc.allow_low_precision(reason="bf16/fp8 attention datapath"):
      for _rep in range(reps):
        with tc.tile_pool(name="consts", bufs=1) as consts, \
             tc.tile_pool(name="persist", bufs=1) as persist, \
             tc.tile_pool(name="persist2", bufs=2) as persist2, \
             tc.tile_pool(name="rpool",
                          bufs=(4 if "rp4" in opts else 3)) as rpool, \
             tc.tile_pool(name="ptile",
                          bufs=(8 if "pp8" in opts else 6)) as ppool, \
             tc.tile_pool(name="small",
                          bufs=(4 if "sm4" in opts else 3)) as smallpool, \
             tc.tile_pool(name="outsb", bufs=3) as outpool, \
             tc.tile_pool(name="slots", bufs=3, space="PSUM") as slotp, \
             tc.tile_pool(name="otp", bufs=1, space="PSUM") as otp:

            mm = lambda *a, **k: nc.tensor.matmul(*a, **k)

            # ---- constants + resident inputs ------------------------------
            Wq_sb = consts.tile([128, NJ, H * 128], MDT, tag="Wq")
            Wk_sb = consts.tile([128, NJ, H * 128], MDT, tag="Wk")
            Wv_sb = consts.tile([128, NJ, D], MDT, tag="Wv")
            Wo_sb = consts.tile([128, NJ, D], MDT, tag="Wo")
            kin = consts.tile([128, NJ, KP], MDT, tag="kin")
            qin = consts.tile([128, NJ, QL], MDT, tag="qin")
            vin = consts.tile([128, NJ, KP], MDT, tag="vin")
            bq_sb = consts.tile([128, H], F32, tag="bq")
            bk_sb = consts.tile([128, H], F32, tag="bk")
            bvb_sb = consts.tile([128, D], F32, tag="bvb")
            bob_sb = consts.tile([128, D], F32, tag="bob")
            npad_sb = consts.tile([1, 1], F32, tag="npad")
            # k/q-path inputs first (SP queue is FIFO; these gate the first
            # scores), then the v path, then out-proj weights + epilogue.
            # k/q path on the SP DGE queue, v path + epilogue consts on the
            # ACT DGE queue (idle until the first exp) - halves the serial
            # dispatch chain gating the first scores.
            nc.sync.dma_start(
                Wk_sb[:], Wk_d.rearrange("(j p) n -> p j n", p=128)
            )
            kin_src = kT_d.rearrange("(j p) n -> p j n", p=128)
            qin_src = qT_d.rearrange("(j p) n -> p j n", p=128)
            nc.sync.dma_start(kin[:, :, 0:512], kin_src[:, :, 0:512])
            nc.sync.dma_start(kin[:, :, 512:1024], kin_src[:, :, 512:1024])
            nc.sync.dma_start(
                Wq_sb[:], Wq_d.rearrange("(j p) n -> p j n", p=128)
            )
            nc.sync.dma_start(qin[:, :, 0:512], qin_src[:, :, 0:512])
            nc.sync.dma_start(bk_sb[:], bk_d[:])
            nc.sync.dma_start(bq_sb[:], bq_d[:])
            nc.sync.dma_start(npad_sb[:], npad_d[:])
            for s0 in range(1024, KP, 512):
                nc.sync.dma_start(
                    kin[:, :, s0:s0 + 512], kin_src[:, :, s0:s0 + 512]
                )
            for s0 in range(512, QL, 512):
                nc.sync.dma_start(
                    qin[:, :, s0:s0 + 512], qin_src[:, :, s0:s0 + 512]
                )
            nc.scalar.dma_start(
                vin[:], vT_d.rearrange("(j p) n -> p j n", p=128)
            )
            nc.scalar.dma_start(
                Wv_sb[:], Wv_d.rearrange("(j p) n -> p j n", p=128)
            )
            nc.scalar.dma_start(bvb_sb[:], bvb_d[:])
            nc.scalar.dma_start(
                Wo_sb[:], Wo_d.rearrange("(j p) n -> p j n", p=128)
            )
            nc.scalar.dma_start(bob_sb[:], bob_d[:])
            ones_sb = consts.tile([1, DH], MDT, tag="ones")
            nc.vector.memset(ones_sb[:], 1.0)
            ebias_sb = consts.tile([128, 1], F32, tag="ebias")
            nc.vector.memset(ebias_sb[:], EXP_SHIFT)
            xT_sb = persist.tile([128, NJ, QL], MDT, tag="xT_sb")

            def new_slot():
                return slotp.tile([128, QL], F32, tag="slot", name="slot")

            # ---- per-head projections straight into kTr/qTr layouts -------
            # Wkr/Wqr columns are host-replicated per head (M=128 = the
            # head's 64 dims twice), so the projection psum holds the head's
            # dims in both partition halves; DVE then writes kt-even tiles
            # to partitions 0:64 and kt-odd to 64:128 (k), or copies straight
            # through (q replicated). No partition-crossing moves anywhere.
            def make_head(c, hh):
                h = 2 * c + hh
                hs = slice(h * 128, (h + 1) * 128)
                st = {
                    "kTr": rpool.tile([128, NE * 128], MDT, tag="kTr",
                                      name="kTr"),
                    "qTr": rpool.tile([128, QL], MDT, tag="qTr", name="qTr"),
                }
                units = []

                def k_unit(s0):
                    def run():
                        ps = new_slot()[:, :512]
                        sl = min(512, KP - s0)
                        for j in range(NJ):
                            mm(
                                ps[:, :sl],
                                Wk_sb[:, j, hs],
                                kin[:, j, s0:s0 + sl],
                                start=(j == 0),
                                stop=(j == NJ - 1),
                            )
                        t0 = s0 // 128
                        tiles = list(range(t0, min(t0 + 4, NKT)))
                        for po, par in ((0, 0), (DH, 1)):
                            tl = [t for t in tiles if t % 2 == par]
                            if not tl:
                                continue
                            e0 = tl[0] // 2
                            src = ps[po:po + DH, (tl[0] - t0) * 128:].rearrange(
                                "p (u n) -> p u n", n=128
                            )[:, 0:2 * len(tl) - 1:2, :]
                            nc.vector.tensor_scalar_add(
                                st["kTr"][po:po + DH,
                                          e0 * 128:(e0 + len(tl)) * 128]
                                .rearrange("p (u n) -> p u n", n=128),
                                src,
                                bk_sb[po:po + DH, h:h + 1],
                            )
                    return run

                k_units = [k_unit(s0) for s0 in range(0, KP, 512)]

                def q_unit(s0):
                    def run():
                        ps = new_slot()[:, :512]
                        for j in range(NJ):
                            mm(
                                ps[:],
                                Wq_sb[:, j, hs],
                                qin[:, j, s0:s0 + 512],
                                start=(j == 0),
                                stop=(j == NJ - 1),
                            )
                        nc.vector.tensor_scalar_add(
                            st["qTr"][:, s0:s0 + 512], ps[:],
                            bq_sb[:, h:h + 1],
                        )
                    return run

                q_units = [q_unit(s0) for s0 in range(0, QL, 512)]
                units.append(k_units.pop(0))
                units.append(q_units.pop(0))
                while k_units or q_units:
                    if q_units:
                        units.append(q_units.pop(0))
                    if k_units:
                        units.append(k_units.pop(0))
                done = [False]

                def mark():
                    done[0] = True

                units.append(mark)
                return st, units, done

            # ---- per-block v projection -----------------------------------
            def make_vblock(c):
                cs = slice(c * 128, (c + 1) * 128)
                v_c = persist2.tile([128, NKT, 2, 72], PV_DT, tag="v",
                                    name="v_c")
                vready = [0]
                units = []

                def memset_unit():
                    nc.vector.memset(v_c[:, :, :, DH:DH + 1], 1.0)
                    nc.vector.memset(v_c[:, :, :, DH + 1:], 0.0)

                units.append(memset_unit)

                def v_unit(t):
                    def run():
                        ps = new_slot()[:, :512]
                        for j in range(NJ):
                            mm(
                                ps[:, 0:128],
                                vin[:, j, t * 128:(t + 1) * 128],
                                Wv_sb[:, j, cs],
                                start=(j == 0),
                                stop=(j == NJ - 1),
                            )
                        nc.vector.tensor_add(
                            v_c[:, t, :, 0:DH],
                            ps[:, 0:128].rearrange("p (h d) -> p h d", h=2),
                            bvb_sb[:, cs].rearrange("p (h d) -> p h d", h=2),
                        )
                        vready[0] = t + 1
                    return run

                for t in range(NKT):
                    units.append(v_unit(t))
                return v_c, units, vready

            def drain(units, n=None):
                k = len(units) if n is None else min(n, len(units))
                for _ in range(k):
                    units.pop(0)()

            # ---- deferred per-head normalization --------------------------
            deferred_norm = [None]

            def make_norm(OT, c, po):
                r_box = [None]

                def run_dve():
                    li = smallpool.tile([1, QL], F32, tag="li", name="li")
                    nc.vector.tensor_scalar_add(
                        li[:], OT[DH:DH + 1, :], npad_sb[:]
                    )
                    r_sb = smallpool.tile([1, QL], MDT, tag="r", name="r_sb")
                    nc.vector.reciprocal(r_sb[:], li[:])
                    r_box[0] = r_sb

                def run_pe():
                    r_sb = r_box[0]
                    rb = new_slot()[0:DH, :]
                    for q0 in range(0, QL, 512):
                        mm(
                            rb[:, q0:q0 + 512],
                            ones_sb[:],
                            r_sb[:, q0:q0 + 512],
                            start=True, stop=True,
                        )
                    rb_sb = smallpool.tile([DH, QL], F32, tag="rb_sb",
                                           name="rb_sb")
                    nc.vector.tensor_copy(rb_sb[:], rb[:])
                    nc.vector.tensor_mul(
                        xT_sb[po:po + DH, c, :], OT[0:DH, :], rb_sb[:]
                    )
                return (run_dve, run_pe)

            # ---- attention per head, with interleaved work units ----------
            def attend(hst, v_c, vready, c, hh, units):
                def v_gate(need):
                    while vready[0] < min(need, NKT) and units:
                        drain(units, 1)
                po = hh * DH
                kTr, qTr = hst["kTr"], hst["qTr"]
                OT = otp.tile([DH + 1, QL], F32, tag="OT", name="OT")

                def emit_scores(j):
                    sa, sb = new_slot(), new_slot()
                    for q0 in range(0, QL, 512):
                        mm(
                            sa[:, q0:q0 + 512],
                            kTr[0:DH, j * 128:(j + 1) * 128],
                            qTr[0:DH, q0:q0 + 512],
                            start=True, stop=True,
                        )
                        mm(
                            sb[:, q0:q0 + 512],
                            kTr[DH:128, j * 128:(j + 1) * 128],
                            qTr[DH:128, q0:q0 + 512],
                            start=True, stop=True,
                        )
                    return sa, sb

                def emit_exp(j, sa, sb):
                    P2 = ppool.tile([128, 2, QL], PV_DT, tag="P", name="P")
                    nc.scalar.activation(
                        P2[:, 0, :], sa[:], EXP,
                        bias=ebias_sb[:], scale=0.125,
                    )
                    nc.scalar.activation(
                        P2[:, 1, :], sb[:], EXP,
                        bias=ebias_sb[:], scale=0.125,
                    )
                    return P2

                def emit_pv(j, P2):
                    if USE_DR:
                        v2 = v_c[:, 2 * j:2 * j + 2, hh, 0:DH + 1]
                        for q0 in range(0, QL, 256):
                            mm(
                                OT[:, q0:q0 + 256],
                                v2,
                                P2[:, :, q0:q0 + 256],
                                start=(j == 0 and q0 % 512 == 0),
                                stop=(j == NPAIR - 1 and TAIL == 0
                                      and q0 % 512 == 256),
                                perf_mode=DR,
                            )
                        return
                    for i in range(2):
                        t = 2 * j + i
                        for q0 in range(0, QL, 512):
                            mm(
                                OT[:, q0:q0 + 512],
                                v_c[:, t, hh, 0:DH + 1],
                                P2[:, i, q0:q0 + 512],
                                start=(t == 0),
                                stop=(t == NKT - 1 and TAIL == 0),
                            )

                def emit_tail():
                    t = NKT - 1
                    Tt = new_slot()
                    for q0 in range(0, QL, 512):
                        mm(
                            Tt[:, q0:q0 + 512],
                            kTr[0:DH, NPAIR * 128:NE * 128],
                            qTr[0:DH, q0:q0 + 512],
                            start=True, stop=True,
                        )
                    Pt = ppool.tile([128, 2, QL], PV_DT, tag="P", name="Pt")
                    nc.scalar.activation(
                        Pt[:, 0, :], Tt[:], EXP,
                        bias=ebias_sb[:], scale=0.125,
                    )
                    for q0 in range(0, QL, 512):
                        mm(
                            OT[:, q0:q0 + 512],
                            v_c[:, t, hh, 0:DH + 1],
                            Pt[:, 0, q0:q0 + 512],
                            start=(NPAIR == 0),
                            stop=True,
                        )

                pend = {}
                for j in range(NPAIR):
                    sa, sb = emit_scores(j)
                    pend[j] = emit_exp(j, sa, sb)
                    if j == 0 and deferred_norm[0] is not None:
                        deferred_norm[0][0]()
                    if j == min(3, NPAIR - 1) and deferred_norm[0] is not None:
                        deferred_norm[0][1]()
                        deferred_norm[0] = None
                    if j > 0:
                        v_gate(2 * j)
                        emit_pv(j - 1, pend.pop(j - 1))
                    if "fl" in opts:
                        drain(units, 4 if j < NPAIR - 2 else 0)
                    else:
                        drain(units, 3)
                if NPAIR:
                    v_gate(2 * NPAIR)
                    emit_pv(NPAIR - 1, pend.pop(NPAIR - 1))
                if TAIL:
                    drain(units)
                    emit_tail()
                deferred_norm[0] = make_norm(OT, c, po)

            # ---- main schedule --------------------------------------------
            # heads are projected one ahead; v blocks one block ahead.
            hst, hunits, _ = make_head(0, 0)
            drain(hunits)
            v_c, vunits, vready = make_vblock(0)
            nxt_h = None
            nxt_v = None
            for c in range(NJ):
                for hh in range(2):
                    units = []
                    if hh == 0:
                        nxt_h, nunits, _ = make_head(c, 1)
                        units.extend(vunits)  # v units lead: pv needs them
                        units.extend(nunits)
                        vunits = []
                    else:
                        if c + 1 < NJ:
                            nxt_h, nunits, _ = make_head(c + 1, 0)
                            units.extend(nunits)
                            nxt_v, nvunits, nvready = make_vblock(c + 1)
                            units.extend(nvunits)
                    attend(hst, v_c, vready, c, hh, units)
                    drain(units)
                    hst, nxt_h = nxt_h, None
                if nxt_v is not None:
                    v_c, vready, nxt_v = nxt_v, nvready, None

            deferred_norm[0][0]()
            deferred_norm[0][1]()
            deferred_norm[0] = None

            # ---- output projection ----------------------------------------
            for t in range(QL // 128):
                ps = new_slot()[:, :512]
                for cc in range(NJ):
                    mm(
                        ps[:],
                        xT_sb[:, cc, t * 128:(t + 1) * 128],
                        Wo_sb[:, cc, :],
                        start=(cc == 0),
                        stop=(cc == NJ - 1),
                    )
                osb = outpool.tile([128, D], F32, tag="osb", name="osb")
                nc.vector.tensor_add(osb[:], ps[:], bob_sb[:])
                nc.sync.dma_start(out_d[t * 128:(t + 1) * 128, :], osb[:])

    nc.compile()
    return nc


def prep_inputs3(query, key_in, value, mask, Wq, bq, Wk, bk, Wv, bv, Wo, bo):
    """Host-side shard/compact/transpose. Returns (in_maps, NKT)."""
    import ml_dtypes

    actd = ml_dtypes.bfloat16
    query = np.ascontiguousarray(np.asarray(query, np.float32))
    key_in = np.ascontiguousarray(np.asarray(key_in, np.float32))
    value = np.ascontiguousarray(np.asarray(value, np.float32))
    mask = np.asarray(mask)
    bq = np.asarray(bq, np.float32)
    bk = np.asarray(bk, np.float32)
    bv = np.asarray(bv, np.float32)
    bo = np.asarray(bo, np.float32)

    idx = [np.nonzero(mask[b] != 0)[0] for b in range(B)]
    counts = [len(ix) for ix in idx]
    NKT = max(1, _ceil_div(max(counts), 128))
    KP = NKT * 128

    kT = np.zeros((B, D, KP), np.float32)
    vT = np.zeros((B, D, KP), np.float32)
    for b in range(B):
        kT[b, :, :counts[b]] = key_in[b, idx[b]].T
        vT[b, :, :counts[b]] = value[b, idx[b]].T
    qT = np.ascontiguousarray(query.transpose(0, 2, 1))  # [B, D, S]

    bvb = np.ascontiguousarray(np.broadcast_to(bv, (128, D)))
    bob = np.ascontiguousarray(np.broadcast_to(bo, (128, D)))

    kTc = [np.ascontiguousarray(kT[b], actd) for b in range(B)]
    vTc = [np.ascontiguousarray(vT[b], actd) for b in range(B)]

    def repl(W):
        # [D, D] -> [D, H, 2, 64] -> [D, H*128]: each head's 64 output dims
        # duplicated so the projection psum fills both partition halves.
        Wr = np.repeat(
            np.asarray(W, np.float32).reshape(D, H, 1, DH), 2, axis=2
        ).reshape(D, H * 128)
        return np.ascontiguousarray(Wr, actd)

    Wqc = repl(Wq)
    Wkc = repl(Wk)
    Wvc = np.ascontiguousarray(Wv, actd)
    Woc = np.ascontiguousarray(Wo, actd)
    bq2r = np.ascontiguousarray(
        np.repeat(bq.reshape(H, 1, DH), 2, axis=1).reshape(H, 128).T
    )
    bk2r = np.ascontiguousarray(
        np.repeat(bk.reshape(H, 1, DH), 2, axis=1).reshape(H, 128).T
    )
    npadn = [
        np.full((1, 1), -(KP - counts[b]) * np.exp(EXP_SHIFT), np.float32)
        for b in range(B)
    ]
    in_maps = []
    for core in range(NCORES):
        b, r = divmod(core, QSPLIT)
        in_maps.append({
            "qT": np.ascontiguousarray(qT[b, :, r * QL:(r + 1) * QL], actd),
            "kT": kTc[b],
            "vT": vTc[b],
            "Wqr": Wqc, "Wkr": Wkc, "Wv": Wvc, "Wo": Woc,
            "bq2r": bq2r, "bk2r": bk2r, "bvb": bvb, "bob": bob,
            "npadn": npadn[b],
        })
    return in_maps, NKT


def _get_nc3(NKT):
    key = ("v3", NKT)
    if key not in _nc_cache:
        _nc_cache[key] = build_program3(NKT)
    return _nc_cache[key]


def _assemble(results):
    out = np.empty((B, S, D), np.float32)
    for core in range(NCORES):
        b, r = divmod(core, QSPLIT)
        out[b, r * QL:(r + 1) * QL] = results[core]["out"]
    return out


def kernel(query, key_in, value, mask, Wq, bq, Wk, bk, Wv, bv, Wo, bo):
    in_maps, NKT = prep_inputs3(
        query, key_in, value, mask, Wq, bq, Wk, bk, Wv, bv, Wo, bo
    )
    nc = _get_nc3(NKT)
    res = run_bass_kernel_spmd(nc, in_maps, list(range(NCORES)))
    return _assemble(res.results)
